# revision 42
# baseline (speedup 1.0000x reference)
"""Trainium2 Bass kernel: attention with rotary embedding + XL memory.

Model (B=2, T=1024, D=2048, H=16, hd=128, XL=1024):
  qkv = x @ w_qkv.T ; split q,k,v ; k_xl += pos_emb ; rope(q), rope(k)
  per head: scores = q @ [k_xl | k].T / sqrt(hd) ; softmax ; y = P @ [v_xl | v]
  out = y @ w_proj.T
Sharding: 8 cores = 2 batches x 4 head-groups (4 heads each); host sums the
4 partial output projections per batch and concatenates batches.

Device design (v3):
  - PE sequencer dispatch is a first-class constraint: a 512-col matmul takes
    213ns on the engine but ~168ns of SEQ with a self-loading fp32r stationary
    vs ~225ns as an Ldweights+Matmult pair (16-bit stationary). Walrus rejects
    mixed 16/32-bit matmul inputs, so dtypes go per matmul family: the scores
    family (kxl, roped q/k) is all-fp32r to keep the attention slots
    SEQ-feasible; everything else (x, wqk, wv, pt, acc, ysb, wproj, ones) is
    fp16 (full PE rate, halves DMA + SBUF, 2x DVE softmax adds).
  - Weights load once (f-outer phase 1, x resident for both t-blocks). DMA
    issue order matches consumption; kxl/vxl/wproj ride late SP queue slots,
    pos_emb accumulates into kxl via Pool-engine accum-DMA, output stores
    (2 obs, 512KB) issue from Pool (SWDGE path, no HWDGE mutex).
  - PE p-state: after any idle the PE runs at 1.2GHz for 3us, so junk warmup
    matmuls cover the DMA-bound prologue and den-latency windows.
  - The v-GEMM streams 512-wide into [t,d] layout (16 mm per t-chunk) as
    PE fillers inside attn-tb0 slots; proj-tb0 blocks fill attn-tb1 slots;
    softmax denominators are two-phase (reduce+recip | bcast+normalize)
    woven around phase boundaries.
"""
import sys

sys.path.insert(0, "/opt/trn_rl_repo")

import numpy as np

import concourse.bass as bass  # noqa: F401
import concourse.mybir as mybir
import concourse.tile as tile
from concourse import bacc
from concourse.bass import ts
from concourse.bass_utils import run_bass_kernel_spmd  # noqa: F401 (fallback)

F32 = mybir.dt.float32
F32R = mybir.dt.float32r
F16 = mybir.dt.float16
AF = mybir.ActivationFunctionType
ADD = mybir.AluOpType.add

B, T, D = 2, 1024, 2048
H, HD, XL = 16, 128, 1024
HPC = 4                 # heads per core
CPB = 4                 # cores per batch
NCORES = 8
NCC = D // 128          # 16 contraction chunks
SCALE = 1.0 / np.sqrt(HD)

_CACHE: dict = {}


def _build_nc():
    nc = bacc.Bacc("TRN2", target_bir_lowering=False, debug=False)

    x_d = nc.dram_tensor("x", [2, 128, NCC, 512], F16, kind="ExternalInput")
    wqk_d = nc.dram_tensor("wqk", [8, 128, NCC, 128], F16,
                           kind="ExternalInput")
    wv_d = nc.dram_tensor("wv", [128, NCC, 512], F16, kind="ExternalInput")
    cs_d = nc.dram_tensor("cs", [128, 2, T], F16, kind="ExternalInput")
    kxl_d = nc.dram_tensor("kxl", [128, 4, XL], F32R, kind="ExternalInput")
    pos_d = nc.dram_tensor("pos", [128, 4, XL], F32R, kind="ExternalInput")
    vxl_d = nc.dram_tensor("vxl", [128, 8, 512], F16, kind="ExternalInput")
    wproj_d = nc.dram_tensor("wproj", [4, 128, 4, 4, 128], F16,
                             kind="ExternalInput")
    out_d = nc.dram_tensor("out", [4, 2, 2, 128, 2, 512], F32,
                           kind="ExternalOutput")

    with tile.TileContext(nc) as tc, nc.allow_low_precision(
            reason="fp16/fp32r matmul inputs and softmax intermediates"):
        with (
            tc.tile_pool(name="const", bufs=1) as const,
            tc.tile_pool(name="wqkp", bufs=3) as wqkp,
            tc.tile_pool(name="wpp", bufs=4) as wpp,
            tc.tile_pool(name="ptp", bufs=8) as ptp,
            tc.tile_pool(name="ropep", bufs=2) as ropep,
            tc.tile_pool(name="accp", bufs=4) as accp,
            tc.tile_pool(name="smallp", bufs=2) as smallp,
            tc.tile_pool(name="outp", bufs=4) as outp,
            tc.tile_pool(name="psum", bufs=3, space="PSUM") as psum,
            tc.tile_pool(name="pyp", bufs=4, space="PSUM") as pyp,
            tc.tile_pool(name="warmp", bufs=1, space="PSUM") as warmp,
        ):
            # ---- persistent tiles ----
            cst = const.tile([128, 2, T], F16, tag="cst")  # [cos;cos],[-s;+s]
            qkq = const.tile([128, 4, T], F32R, tag="qkq")  # roped q.T
            qkk = const.tile([128, 4, T], F32R, tag="qkk")  # roped k.T
            vsb = const.tile([128, 8, 512], F16, tag="vsb")  # v [t, d]
            ysb = const.tile([128, 4, T], F16, tag="ysb")    # y.T per head
            xt0 = const.tile([128, NCC, 512], F16, tag="xt0")
            xt1 = const.tile([128, NCC, 512], F16, tag="xt1")
            wv = const.tile([128, NCC, 512], F16, tag="wv")
            kxl = const.tile([128, 4, XL], F32R, tag="kxl")
            vxl = const.tile([128, 8, 512], F16, tag="vxl")
            ones = const.tile([128, 256], F16, tag="ones")
            xts = (xt0, xt1)
            cc = cst[:, 0, :]
            ss = cst[:, 1, :]

            nc.vector.memset(ones[:], 1.0)

            # ---- DMA prologue (SP issue order = consumption order) ----
            wqk_tiles = {}
            def load_wqk(f):
                wt = wqkp.tile([128, NCC, 128], F16, tag="wqk",
                               name=f"wt{f}")
                nc.sync.dma_start(wt[:], wqk_d[f])
                wqk_tiles[f] = wt
            load_wqk(0)
            nc.sync.dma_start(xt0[:, 0:4, :], x_d[0, :, 0:4, :])
            nc.sync.dma_start(xt0[:, 4:8, :], x_d[0, :, 4:8, :])
            nc.sync.dma_start(cst[:], cs_d[:])
            nc.sync.dma_start(xt0[:, 8:12, :], x_d[0, :, 8:12, :])
            nc.sync.dma_start(xt0[:, 12:16, :], x_d[0, :, 12:16, :])
            load_wqk(1)
            for q in range(4):
                nc.sync.dma_start(xt1[:, 4 * q:4 * q + 4, :],
                                  x_d[1, :, 4 * q:4 * q + 4, :])
            load_wqk(2)
            nc.sync.dma_start(wv[:], wv_d[:])

            # ---- PE warmup: ramp the p-state while DMAs land ----
            def junk(n):
                warm = warmp.tile([128, 512], F32, tag="warm", name="warm")
                for _ in range(n):
                    nc.tensor.matmul(warm[:, 0:256], ones[:, 0:128],
                                     ones[:], start=True, stop=True)
            junk(14)

            # ---- phase 1: QK projection (+rope), weights loaded once ----
            wp_tiles = {}
            def phase1_block(f, tb):
                wt = wqk_tiles[f]
                tbsl = ts(tb, 512)
                pmm = psum.tile([128, 512], F32, tag="ps", name="pmm")
                for ci in range(NCC):
                    nc.tensor.matmul(pmm[:], wt[:, ci, :],
                                     xts[tb][:, ci, :],
                                     start=(ci == 0), stop=(ci == NCC - 1))
                # packed rope: new = P*[cos;cos] + swap(P)*[-sin;+sin]
                sw = ropep.tile([128, 512], F32, tag="sw")
                nc.scalar.copy(sw[0:64, :], pmm[64:128, :])
                nc.scalar.copy(sw[64:128, :], pmm[0:64, :])
                dst = qkq[:, f, tbsl] if f < 4 else qkk[:, f - 4, tbsl]
                t2 = ropep.tile([128, 512], F32, tag="t2")
                nc.vector.tensor_mul(dst, pmm[:], cc[:, tbsl])
                nc.vector.tensor_mul(t2[:], sw[:], ss[:, tbsl])
                nc.vector.tensor_add(dst, dst, t2[:])

            order = [(0, 0), (1, 0), (0, 1), (1, 1)] + [
                (f, tb) for f in range(2, 8) for tb in range(2)]
            for i, (f, tb) in enumerate(order[:KNBLK]):
                phase1_block(f, tb)
                if i == 0:
                    junk(3)
                if tb == 1 and KWQV:
                    if f + 3 < 8:
                        load_wqk(f + 3)
                    if f == 2:
                        nc.sync.dma_start(kxl[:], kxl_d[:])
                        # pos accum rides Pool SWDGE (waits on the kxl load);
                        # per-j slices: whole-tile accum (16KB descriptors)
                        # faults the DMA RMW path on hardware
                        for j in range(4):
                            nc.gpsimd.dma_start(kxl[:, j, :], pos_d[:, j, :],
                                                accum_op=ADD)
                    elif f == 3:
                        nc.sync.dma_start(vxl[:], vxl_d[:])
                    elif f == 5:
                        for g in range(4):
                            wpt = wpp.tile([128, 4, 4, 128], F16, tag="wp",
                                           name=f"wpt{g}")
                            nc.sync.dma_start(wpt[:], wproj_d[g])
                            wp_tiles[g] = wpt

            # ---- v projection: streamed mm-granular PE filler ----
            class VEmit:
                def __init__(self):
                    self.jobs = [(tb, tt) for tb in range(2)
                                 for tt in range(4)]
                    self.g = 0
                    self.ci = 0
                    self.pv = None
                def emit(self, n):
                    while n > 0 and self.g < 8:
                        tb, tt = self.jobs[self.g]
                        if self.ci == 0:
                            self.pv = psum.tile([128, 512], F32, tag="ps",
                                                name="pv")
                        take = min(n, NCC - self.ci)
                        for c in range(self.ci, self.ci + take):
                            nc.tensor.matmul(
                                self.pv[:], xts[tb][:, c, ts(tt, 128)],
                                wv[:, c, :],
                                start=(c == 0), stop=(c == NCC - 1))
                        self.ci += take
                        n -= take
                        if self.ci == NCC:
                            nc.scalar.copy(vsb[:, tb * 4 + tt, :], self.pv[:])
                            self.g += 1
                            self.ci = 0
            vem = VEmit()
            v_fillers = [(lambda n=(10 if s < 8 else 6): vem.emit(n))
                         for s in range(16)]

            # ---- attention ----
            def make_den(tb, h, py, acc):
                """Two-phase denominator: A = partition-reduce + reciprocal,
                B = broadcast + normalize."""
                st = {}
                def den_a():
                    pden_t = psum.tile([128, 512], F32, tag="ps", name="pden")
                    nc.tensor.matmul(pden_t[0:1, :], ones[:, 0:1], acc[:],
                                     start=True, stop=True)
                    rec = smallp.tile([1, 512], F16, tag="rec", bufs=4)
                    nc.vector.reciprocal(rec[:], pden_t[0:1, :])
                    st["rec"] = rec
                def den_b():
                    tbsl = ts(tb, 512)
                    pbc = psum.tile([128, 512], F32, tag="ps", name="pbc")
                    nc.tensor.matmul(pbc[:], ones[0:1, 0:128], st["rec"][:],
                                     start=True, stop=True)
                    rbc = smallp.tile([128, 512], F16, tag="rbc")
                    # tb0: rbc on DVE (ACT is exp-saturated at the quad
                    # boundary); tb1: rbc on ACT (idle, DVE is the den tail)
                    if tb == 0:
                        nc.vector.tensor_copy(rbc[:], pbc[:])
                    else:
                        nc.scalar.copy(rbc[:], pbc[:])
                    nc.vector.tensor_mul(ysb[:, h, tbsl], py[:], rbc[:])
                return den_a, den_b

            def attn_quad(tb, fillers):
                """Chunk-interleaved attention for 4 heads; one filler per
                chunk slot (emitted at the top of the slot)."""
                tbsl = ts(tb, 512)
                py, acc = {}, {}
                for h in range(4):
                    py[h] = pyp.tile([128, 512], F32, tag="py", name=f"py{h}")
                    acc[h] = accp.tile([128, 512], F16, tag="acc",
                                       name=f"acc{h}")
                fill = list(fillers)
                pend = {}
                def emit_av(h):
                    pt_, lv_, kc_ = pend.pop(h)
                    nc.tensor.matmul(py[h][:], lv_, pt_[:],
                                     start=(kc_ == 0), stop=(kc_ == 15))
                for kc in range(16):
                    if fill:
                        fill.pop(0)()
                    for h in range(4):
                        if kc < 8:
                            lk = kxl[:, h, ts(kc, 128)]
                            lv = vxl[:, kc, ts(h, 128)]
                        else:
                            lk = qkk[:, h, ts(kc - 8, 128)]
                            lv = vsb[:, kc - 8, ts(h, 128)]
                        pss = psum.tile([128, 512], F32, tag="ps")
                        nc.tensor.matmul(pss[:], lk, qkq[:, h, tbsl],
                                         start=True, stop=True)
                        pt = ptp.tile([128, 512], F16, tag="pt")
                        nc.scalar.activation(pt[:], pss[:], AF.Exp,
                                             scale=SCALE)
                        if kc == 0:
                            nc.vector.tensor_copy(acc[h][:], pt[:])
                        else:
                            nc.vector.tensor_add(acc[h][:], acc[h][:], pt[:])
                        if h in pend:
                            emit_av(h)
                        pend[h] = (pt, lv, kc)
                for h in range(4):
                    emit_av(h)
                while fill:
                    fill.pop(0)()
                return [make_den(tb, h, py[h], acc[h]) for h in range(4)]

            # ---- output projection (2 obs per store) ----
            # stores alternate Pool SWDGE / SP HWDGE so neither queue's
            # issue+transfer serialization backpressures the ot ring
            ot_state = {}
            def proj(ob, tb):
                g, obi = ob // 4, ob % 4
                # proj-tb0 fillers accumulate in the warm bank so the "ps"
                # ring stays dedicated to the concurrent attention scores
                ptag = "warm" if tb == 0 else "ps"
                pool = warmp if tb == 0 else psum
                po = pool.tile([128, 512], F32, tag=ptag, name="po")
                for yc in range(4):
                    nc.tensor.matmul(po[:], wp_tiles[g][:, obi, yc, :],
                                     ysb[:, yc, ts(tb, 512)],
                                     start=(yc == 0), stop=(yc == 3))
                oh, oi = obi // 2, obi % 2
                if oi == 0:
                    ot_state[tb] = outp.tile([128, 2, 512], F32, tag="ot",
                                             name=f"ot{g}_{oh}_{tb}")
                ot = ot_state[tb]
                nc.vector.tensor_copy(ot[:, oi, :], po[:])
                eng = nc.gpsimd if (g * 2 + oh) % 2 == 0 else nc.sync
                if tb == 1 and g == 3:
                    # final group: 1-ob stores launch as each copy lands so
                    # the drain isn't gated on a big serialized transfer
                    eng = nc.sync if obi % 2 == 0 else nc.gpsimd
                    eng.dma_start(out_d[g, tb, oh][:, oi:oi + 1, :],
                                  ot[:, oi:oi + 1, :])
                elif oi == 1:
                    eng.dma_start(out_d[g, tb, oh], ot[:])

            import os
            KSTAGE = int(os.environ.get("KSTAGE", "0"))
            def flush():
                fo = outp.tile([128, 2, 512], F32, tag="ot", name="flush")
                nc.vector.memset(fo[:], 0.0)
                nc.sync.dma_start(out_d[0, 0, 0], fo[:])
            def stages():
                if KSTAGE == 1:
                    return flush()
                den0 = attn_quad(0, v_fillers if KSTAGE != 2 else [])
                if KSTAGE == 2:
                    return flush()
                if KSTAGE == 3:
                    for h in range(4):
                        den0[h][0]()
                    for h in range(4):
                        den0[h][1]()
                    return flush()
                # den-tb0: reduces interleaved with junk (covers reciprocal
                # latency, spreads psum-ring pressure), broadcasts as early
                # fillers of attn-tb1
                for h in range(4):
                    den0[h][0]()
                    junk(3)
                junk(2)
                den0[0][1]()
                proj0 = [(lambda ob=ob: proj(ob, 0)) for ob in range(16)]
                if KSTAGE == 4:
                    attn_quad(1, [den0[1][1], den0[2][1], den0[3][1]])
                    return flush()
                den1 = attn_quad(1, [lambda: junk(6), den0[1][1], den0[2][1],
                                     den0[3][1]] + proj0[:12])
                # den-tb1 interleaved with the remaining proj-tb0 blocks
                for h in range(4):
                    den1[h][0]()
                    junk(2)
                proj0[12]()
                den1[0][1]()
                proj0[13]()
                den1[1][1]()
                proj0[14]()
                den1[2][1]()
                proj0[15]()
                den1[3][1]()
                junk(6)   # cover the ysb-tb1 normalize latency
                for ob in range(16):
                    proj(ob, 1)
            stages()

    nc.compile()
    return nc


def _get_nc():
    if "nc" not in _CACHE:
        _CACHE["nc"] = _build_nc()
    return _CACHE["nc"]


_PERM = np.concatenate([np.arange(0, HD, 2), np.arange(1, HD, 2)])
_PP = np.concatenate([_PERM + i * HD for i in range(HPC)])  # per-head-block perm


def make_in_maps(x, cos, sin, k_xl, v_xl, pos_emb, w_qkv, w_proj):
    """Host-side shard + layout prep: one input dict per core."""
    x = np.asarray(x, np.float32)
    cos = np.asarray(cos, np.float32)
    sin = np.asarray(sin, np.float32)
    k_xl = np.asarray(k_xl, np.float32)
    v_xl = np.asarray(v_xl, np.float32)
    pos_emb = np.asarray(pos_emb, np.float32)
    w_qkv = np.asarray(w_qkv, np.float32)
    w_proj = np.asarray(w_proj, np.float32)

    # cs[:, 0] = [cos; cos] ; cs[:, 1] = [-sin; +sin]  (packed-rope factors)
    cs = np.ascontiguousarray(np.stack([
        np.concatenate([cos.T, cos.T], axis=0),
        np.concatenate([-sin.T, sin.T], axis=0),
    ], axis=1)).astype(np.float16)

    in_maps = []
    for c in range(NCORES):
        b, g = c // CPB, c % CPB
        h0 = g * HPC
        cols = slice(h0 * HD, (h0 + HPC) * HD)

        # x: [tb, pi, ci, tl]
        x_arr = np.ascontiguousarray(
            x[b].T.reshape(NCC, 128, 2, 512).transpose(2, 1, 0, 3)
        ).astype(np.float16)
        # w_q/w_k rows for this head group, rope-permuted; [f, pi, ci, fcol]
        wq = w_qkv[0 * D + h0 * HD:0 * D + (h0 + HPC) * HD][_PP]
        wk = w_qkv[1 * D + h0 * HD:1 * D + (h0 + HPC) * HD][_PP]
        wqk_rows = np.concatenate([wq, wk], axis=0)  # [1024, D]
        wqk_arr = np.ascontiguousarray(
            wqk_rows.reshape(8, 128, NCC, 128).transpose(0, 3, 2, 1)
        ).astype(np.float16)
        # w_v rows (unpermuted); [pi, ci, col]
        wv_rows = w_qkv[2 * D + h0 * HD:2 * D + (h0 + HPC) * HD]  # [512, D]
        wv_arr = np.ascontiguousarray(
            wv_rows.reshape(512, NCC, 128).transpose(2, 1, 0)
        ).astype(np.float16)
        # k_xl / pos_emb: permuted cols, transposed; [pi, j, t]
        kxlT = k_xl[b][:, cols][:, _PP].T  # [512, XL]
        kxl_arr = np.ascontiguousarray(
            kxlT.reshape(4, 128, XL).transpose(1, 0, 2))
        posT = pos_emb[:, cols][:, _PP].T
        pos_arr = np.ascontiguousarray(
            posT.reshape(4, 128, XL).transpose(1, 0, 2))
        # v_xl natural; [pi, j, col]
        vxl_arr = np.ascontiguousarray(
            v_xl[b][:, cols].reshape(8, 128, 512).transpose(1, 0, 2)
        ).astype(np.float16)
        # w_proj column block, transposed; [g, pi, obi, yc, ocol]
        wprojT = w_proj[:, cols].T  # [512, D]
        wproj_arr = np.ascontiguousarray(
            wprojT.reshape(4, 128, 16, 128).transpose(2, 1, 0, 3)
            .reshape(4, 4, 128, 4, 128).transpose(0, 2, 1, 3, 4)
        ).astype(np.float16)

        in_maps.append({
            "x": x_arr, "wqk": wqk_arr, "wv": wv_arr, "cs": cs,
            "kxl": kxl_arr, "pos": pos_arr, "vxl": vxl_arr,
            "wproj": wproj_arr,
        })
    return in_maps


def unshard(results):
    """results: 8 dicts with 'out' [4, 2, 2, 128, 2, 512] -> [B, T, D]."""
    out = np.zeros((B, T, D), np.float32)
    for c in range(NCORES):
        b = c // CPB
        # dims (g, tb, oh, p, oi, col): channel = g*512 + oh*256 + oi*128 + p
        outT = np.asarray(results[c]["out"]).transpose(
            0, 2, 4, 3, 1, 5).reshape(D, T)
        out[b] += outT.T
    return out


def _get_runner():
    """Persistent jitted 8-core executable (avoids per-call retrace of the
    bass2jax lowering; the NEFF itself is cached by neuronx-cc)."""
    if "runner" in _CACHE:
        return _CACHE["runner"]
    import jax
    import jax.numpy as jnp
    from jax.sharding import Mesh, PartitionSpec, NamedSharding
    from jax.experimental.shard_map import shard_map
    from concourse.bass2jax import (_bass_exec_p, partition_id_tensor,
                                    install_neuronx_cc_hook)

    nc = _get_nc()
    install_neuronx_cc_hook()
    in_names, out_names, out_avals, zero_shapes = [], [], [], []
    for alloc in nc.m.functions[0].allocations:
        if not isinstance(alloc, mybir.MemoryLocationSet):
            continue
        name = alloc.memorylocations[0].name
        if alloc.kind == "ExternalInput":
            if nc.partition_id_tensor is None or \
                    name != nc.partition_id_tensor.name:
                in_names.append(name)
        elif alloc.kind == "ExternalOutput":
            shape = tuple(alloc.tensor_shape)
            np_dt = mybir.dt.np(alloc.dtype)
            out_names.append(name)
            out_avals.append(jax.core.ShapedArray(shape, np_dt))
            zero_shapes.append((shape, np_dt))
    n_params, n_outs = len(in_names), len(out_names)
    all_in = in_names + out_names
    if nc.partition_id_tensor is not None:
        all_in = all_in + [nc.partition_id_tensor.name]

    def _body(*args):
        operands = list(args)
        if nc.partition_id_tensor is not None:
            operands.append(partition_id_tensor())
        return tuple(_bass_exec_p.bind(
            *operands, out_avals=tuple(out_avals), in_names=tuple(all_in),
            out_names=tuple(out_names), lowering_input_output_aliases=(),
            sim_require_finite=True, sim_require_nnan=True, nc=nc))

    devices = jax.devices()[:NCORES]
    mesh = Mesh(np.asarray(devices), ("core",))
    fn = jax.jit(
        shard_map(_body, mesh=mesh,
                  in_specs=(PartitionSpec("core"),) * (n_params + n_outs),
                  out_specs=(PartitionSpec("core"),) * n_outs,
                  check_rep=False),
        donate_argnums=tuple(range(n_params, n_params + n_outs)),
        keep_unused=True)
    sharding = NamedSharding(mesh, PartitionSpec("core"))
    zfn = jax.jit(
        lambda: tuple(jnp.zeros((NCORES * s[0], *s[1:]), d)
                      for s, d in zero_shapes),
        out_shardings=(sharding,) * n_outs)
    runner = (fn, zfn, in_names, out_names, out_avals, sharding)
    _CACHE["runner"] = runner
    return runner


def kernel(x, cos, sin, k_xl, v_xl, pos_emb, w_qkv, w_proj, is_causal=0,
           **_ignored):
    # is_causal is 0 for this problem spec (fill=arange, shape []); the
    # non-causal path is the only one implemented.
    import jax
    in_maps = make_in_maps(x, cos, sin, k_xl, v_xl, pos_emb, w_qkv, w_proj)
    fn, zfn, in_names, out_names, out_avals, sharding = _get_runner()
    concat_in = [
        jax.device_put(
            np.concatenate([in_maps[c][nm] for c in range(NCORES)], axis=0),
            sharding)
        for nm in in_names]
    outs = fn(*concat_in, *zfn())
    results = [
        {nm: np.asarray(outs[i]).reshape(NCORES, *out_avals[i].shape)[c]
         for i, nm in enumerate(out_names)}
        for c in range(NCORES)]
    _CACHE["last_results"] = None
    return unshard(results)
